# revision 19
# baseline (speedup 1.0000x reference)
"""ExpertsChooseMlp Trainium2 kernel.

Full inputs in, full output out. Sharding: 8 cores = 4 batches x 2 expert-pairs.
Core m handles batch b=m//2 and experts {2g, 2g+1}, g=m%2. Each core computes
pout[T,O] = sum_{e in pair} combine[b,:,e,:] @ mlp_e(dispatch[b,:,e,:]^T @ x[b]);
the host sums the two partials per batch and adds b2.

All matmuls run in bf16 with fp32 PSUM accumulation. Layouts are chosen so the
natural (host-prepared) operand orientations feed the PE directly:
  xdT[D,C] = matmul(lhsT=x[b][T,D],  rhs=dm_e[T,C])     (K=T)
  hT[HE,C] = matmul(lhsT=w1_e[D,HE], rhs=xdT[D,C])      (K=D), then GELU+b1
  y[C,O]   = matmul(lhsT=hT[HE,C],   rhs=w2_e[HE,O])    (K=HE)
  pout[T,O]= matmul(lhsT=cmT_e[C,T], rhs=y[C,O])        (K=C, accum over e)
Only cmT (combine slice transposed) is materialized host-side.
"""
import sys

sys.path.insert(0, "/opt/trn_rl_repo")

import numpy as np
import ml_dtypes

import concourse.bacc as bacc
import concourse.mybir as mybir
import concourse.tile as tile
from concourse import bass_utils

B, T, D, E, C, HE, O = 4, 2048, 512, 4, 1024, 512, 512
P = 128
nKT = T // P      # 16 T-chunks
nMD = D // P      # 4  D-chunks
nMH = HE // P     # 4  HE-chunks
nKD = D // P      # 4
nCC = C // P      # 8  C-chunks
nKH = HE // P     # 4
nMT = T // P      # 16
NF = 512          # matmul free dim (one PSUM bank)

F32 = mybir.dt.float32
BF16 = mybir.dt.bfloat16
F8 = mybir.dt.float8e4
GELU = mybir.ActivationFunctionType.Gelu
DR = mybir.MatmulPerfMode.DoubleRow
nCP = nCC // 2    # 4  C pair-chunks (DoubleRow: K=256 per matmul)

_NC = None


def _build():
    nc = bacc.Bacc("TRN2", target_bir_lowering=False, debug=False,
                   enable_asserts=False, num_devices=8)
    xb = nc.dram_tensor("xb", [T, D], BF16, kind="ExternalInput").ap()
    dm = nc.dram_tensor("dm", [2, T, C], BF16, kind="ExternalInput").ap()
    cmt = nc.dram_tensor("cmt", [2, C, T], F8, kind="ExternalInput").ap()
    w1 = nc.dram_tensor("w1", [2, D, HE], BF16, kind="ExternalInput").ap()
    w2 = nc.dram_tensor("w2", [2, HE, O], BF16, kind="ExternalInput").ap()
    b1 = nc.dram_tensor("b1s", [2, HE], F32, kind="ExternalInput").ap()
    pout = nc.dram_tensor("pout", [T, O], F32, kind="ExternalOutput").ap()

    with tile.TileContext(nc) as tc:
        with (
            tc.tile_pool(name="const", bufs=1) as const,
            tc.tile_pool(name="dmp", bufs=32) as dmp,
            tc.tile_pool(name="cmp", bufs=8) as cmp_,
            tc.tile_pool(name="inter", bufs=1) as inter,
            tc.tile_pool(name="yp", bufs=2) as yp,
            tc.tile_pool(name="outp", bufs=2) as outp,
            tc.tile_pool(name="psum", bufs=8, space="PSUM") as psp,
        ):
            # ---- resident constants (ACT HWDGE ring) ----
            # x split per-chunk so the first matmul isn't gated on a 2MB DMA;
            # the first dispatch-mask tiles ride the same ring interleaved so
            # phase A's head isn't gated on the sync engine's Tile preamble
            x_sb = const.tile([P, nKT, D], BF16)
            dm_head = []
            for kt in range(nKT):
                if kt < 4:
                    t_ = dmp.tile([P, C], BF16, tag="dm", name=f"dmh{kt}")
                    nc.scalar.dma_start(t_[:], dm[0, kt * P:(kt + 1) * P, :])
                    dm_head.append(t_)
                nc.scalar.dma_start(x_sb[:, kt, :], xb[kt * P:(kt + 1) * P, :])
            w1_sb = const.tile([P, 2, nKD, HE], BF16)
            nc.scalar.dma_start(w1_sb[:], w1.rearrange("e (kd p) j -> p e kd j", p=P))
            w2_sb = const.tile([P, 2, nKH, O], BF16)
            nc.scalar.dma_start(w2_sb[:], w2.rearrange("e (kh p) j -> p e kh j", p=P))
            b1_sb = const.tile([P, 2 * nMH], F32)
            nc.scalar.dma_start(b1_sb[:], b1.rearrange("e (mh p) -> p (e mh)", p=P))

            y_tiles = []
            for ei in range(2):
                # ---- dispatch-mask tiles for this expert (SYNC ring; the
                # first 4 of expert 0 were prefetched on the ACT ring) ----
                dm_t = []
                for kt in range(nKT):
                    if ei == 0 and kt < 4:
                        dm_t.append(dm_head[kt])
                        continue
                    t_ = dmp.tile([P, C], BF16, tag="dm")
                    nc.sync.dma_start(t_[:], dm[ei, kt * P:(kt + 1) * P, :])
                    dm_t.append(t_)

                # ---- phase A: xdT[D, C] ----
                # kt-outer: all 8 PSUM banks accumulate in parallel, so each
                # dm tile is consumed once (at sustainable DMA rate) and
                # released immediately for the next expert's prefetch.
                xdt = inter.tile([P, nMD, C], BF16, tag="xdt")
                pss = [psp.tile([P, NF], F32, tag="ps", name=f"psa{i}")
                       for i in range(2 * nMD)]
                for kt in range(nKT):
                    for mc in range(nMD):
                        lhsT = x_sb[:, kt, mc * P:(mc + 1) * P]
                        nc.tensor.matmul(pss[2 * mc][:], lhsT, dm_t[kt][:, 0:NF],
                                         start=(kt == 0), stop=(kt == nKT - 1))
                        nc.tensor.matmul(pss[2 * mc + 1][:], lhsT, dm_t[kt][:, NF:C],
                                         start=(kt == 0), stop=(kt == nKT - 1))
                for ncc in range(2):
                    for mc in range(nMD):
                        nc.vector.tensor_copy(xdt[:, mc, ncc * NF:(ncc + 1) * NF],
                                              pss[2 * mc + ncc][:])

                # ---- phase B: hT[HE, C] = gelu(w1^T xdT + b1) ----
                # ncc-outer so phase C's first C-half unblocks after 4 gelus
                ht = inter.tile([P, nMH, C], BF16, tag="ht")
                for ncc in range(2):
                    sl = slice(ncc * NF, (ncc + 1) * NF)
                    for mh in range(nMH):
                        ps0 = psp.tile([P, NF], F32, tag="ps")
                        for kd in range(nKD):
                            nc.tensor.matmul(ps0[:],
                                             w1_sb[:, ei, kd, mh * P:(mh + 1) * P],
                                             xdt[:, kd, sl],
                                             start=(kd == 0), stop=(kd == nKD - 1))
                        bia = b1_sb[:, ei * nMH + mh:ei * nMH + mh + 1]
                        nc.scalar.activation(ht[:, mh, sl], ps0[:], GELU, bias=bia)

                # ---- phase C: y[C, O] (stored fp8, DoubleRow plane layout:
                # row c = kp*256 + i*128 + p  ->  y_sb[p, kp, i, :]) ----
                y_sb = yp.tile([P, nCP, 2, O], F8, tag="y")
                for cc in range(nCC):
                    ps = psp.tile([P, NF], F32, tag="ps")
                    for kh in range(nKH):
                        nc.tensor.matmul(ps[:], ht[:, kh, cc * P:(cc + 1) * P],
                                         w2_sb[:, ei, kh, :],
                                         start=(kh == 0), stop=(kh == nKH - 1))
                    nc.vector.tensor_copy(y_sb[:, cc // 2, cc % 2, :], ps[:])
                y_tiles.append(y_sb)

            # ---- combine-mask tiles (fp8, [P, plane, T]): SYNC ring behind
            # the dm loads so they can't steal early HBM bandwidth ----
            cmt_t = {}
            for ei in range(2):
                for kp in range(nCP):
                    t_ = cmp_.tile([P, 2, T], F8, tag="cmt")
                    nc.sync.dma_start(
                        t_[:],
                        cmt[ei, kp * 2 * P:(kp + 1) * 2 * P, :]
                        .rearrange("(i p) t -> p i t", p=P))
                    cmt_t[(ei, kp)] = t_

            # ---- phase D: pout[T, O] = sum_e cmT_e^T y_e (fp8 DoubleRow) ----
            for mt in range(nMT):
                ps = psp.tile([P, NF], F32, tag="ps")
                idx = 0
                for ei in range(2):
                    for kp in range(nCP):
                        nc.tensor.matmul(ps[:],
                                         cmt_t[(ei, kp)][:, :, mt * P:(mt + 1) * P],
                                         y_tiles[ei][:, kp, :, :],
                                         start=(idx == 0), stop=(idx == 7),
                                         perf_mode=DR)
                        idx += 1
                if mt % 4 == 0:
                    ot = outp.tile([P, 4, O], F32, tag="out")
                nc.vector.tensor_copy(ot[:, mt % 4, :], ps[:])
                if mt % 4 == 3:
                    nc.sync.dma_start(
                        pout[(mt - 3) * P:(mt + 1) * P, :]
                        .rearrange("(m p) o -> p m o", p=P), ot[:])

    nc.compile()
    return nc


def get_nc():
    global _NC
    if _NC is None:
        _NC = _build()
    return _NC


def make_in_maps(x, dispatch_mask, combine_array, w1, b1, w2):
    bf = ml_dtypes.bfloat16
    in_maps = []
    for m in range(8):
        b, g = m // 2, m % 2
        es = slice(2 * g, 2 * g + 2)
        dm_s = np.ascontiguousarray(
            np.transpose(dispatch_mask[b, :, es, :], (1, 0, 2))).astype(bf)
        cmt_s = np.ascontiguousarray(
            np.transpose(combine_array[b, :, es, :], (1, 2, 0))).astype(
                ml_dtypes.float8_e4m3)
        in_maps.append({
            "xb": np.ascontiguousarray(x[b]).astype(bf),
            "dm": dm_s,
            "cmt": cmt_s,
            "w1": np.ascontiguousarray(w1[es]).astype(bf),
            "w2": np.ascontiguousarray(w2[es]).astype(bf),
            "b1s": np.ascontiguousarray(b1[es]).astype(np.float32),
        })
    return in_maps


def kernel(x, dispatch_mask, combine_array, w1, b1, w2, b2):
    nc = get_nc()
    in_maps = make_in_maps(x, dispatch_mask, combine_array, w1, b1, w2)
    res = bass_utils.run_bass_kernel_spmd(nc, in_maps, core_ids=list(range(8)))
    b2f = np.asarray(b2, dtype=np.float32)
    out = np.empty((B, T, O), dtype=np.float32)
    for b in range(B):
        out[b] = res.results[2 * b]["pout"] + res.results[2 * b + 1]["pout"] + b2f
    return out
